# revision 1
# baseline (speedup 1.0000x reference)
"""CreditRiskGAT on 8 Trainium2 NeuronCores.

3-layer GAT (PyG GATConv semantics, eval mode) + sigmoid regressor.
Strategy: nodes partitioned across 8 cores (6250 each). Edges (with self
loops) are sorted by dst and bucketed to the dst-owning core. Per layer:
  - dense transform runs on the dst owner's nodes
  - per-edge phase: dma_gather of source-node features (x / h2 / h3 tables),
    attention weights w = exp(leakyrelu(e_src+e_dst)) computed on device,
    segment softmax folded into an unnormalized weighted aggregation
    (matmul with a one-hot "selection" matrix built from dst offsets),
    normalization by the gathered weight-sum column.
  - h2/h3 tables are AllGathered in 7 chunks so communication overlaps
    the producing layer's compute.
All floating point math runs on device; the host only builds index arrays.
"""
import sys

sys.path.insert(0, "/opt/trn_rl_repo")

import numpy as np
import ml_dtypes

import concourse.bass as bass
import concourse.bacc as bacc
import concourse.mybir as mybir
import concourse.tile as tile
from concourse.bass_types import AP
from concourse.bass_utils import run_bass_kernel_spmd
from concourse.masks import make_identity

f32 = mybir.dt.float32
bf16 = mybir.dt.bfloat16
i16 = mybir.dt.int16
AF = mybir.ActivationFunctionType
OP = mybir.AluOpType

# problem constants (hardcoded per contract)
N, F_IN, H, C1, C2, C3 = 50000, 66, 8, 128, 128, 64
NCORES, NP = 8, 6250
T = 49                      # node tiles per core (49*128 = 6272)
NPAD = T * 128              # padded nodes per core
B0, B1 = 5, 3               # subchunks per tile for src-table half0/half1
CPT = B0 + B1               # total subchunks (slots = CPT*128 per tile)
GT = 7                      # tiles per gather group
NG = T // GT                # gather groups (= allgather chunks)
CH = GT * 128               # rows per allgather chunk (896)
HALF = 32768                # int16 index range per gather half
NHE = NCORES * NPAD         # rows in allgathered tables (50176)
EDR = 6336                  # ed table rows (>= NPAD+1, 256B-stride space)
NEG_SLOPE = 0.2

_CACHE = {}


# ---------------------------------------------------------------- host side
def _wrap16(vals):
    """dma_gather index layout: element k -> idxs[k % 16, k // 16]."""
    k = len(vals)
    m = np.zeros((16, k // 16), np.int16)
    m[np.arange(k) % 16, np.arange(k) // 16] = vals
    return np.tile(m, (8, 1))


def _edge_plan(src_g, drel_g, tl_g, key, tab_idx):
    """Build per-core slot arrays for one edge ordering.

    src_g:  global gather index per edge (into the layer's src table)
    drel_g: dst offset within node tile (0..127)
    tl_g:   node tile id per edge
    key:    half selector (tab_idx >= HALF)
    Returns (iA, iB, iED, drel) group-gather arrays.
    """
    slots_idx = np.zeros((T, CPT, 128), np.int64)      # table idx per slot
    slots_drel = np.zeros((T, CPT, 128), np.float32)
    slots_ed = np.full((T, CPT, 128), NPAD, np.int64)  # ed row (pad -> NPAD)
    for t in range(T):
        for half in (0, 1):
            m = (tl_g == t) & (key == half)
            n = int(m.sum())
            cap = (B0 if half == 0 else B1) * 128
            assert n <= cap, (t, half, n)
            base = 0 if half == 0 else B0
            idxs = tab_idx[m] - (HALF if half else 0)
            sub = base + np.arange(n) // 128
            par = np.arange(n) % 128
            slots_idx[t, sub, par] = idxs
            slots_drel[t, sub, par] = drel_g[m]
            slots_ed[t, sub, par] = t * 128 + drel_g[m]
    # group-gather index arrays: gathered element k = block*128 + partition,
    # block = (tile_in_group)*nsub + sub -> plain C-order flatten
    iA = np.stack([
        _wrap16(slots_idx[g * GT:(g + 1) * GT, 0:B0, :].reshape(-1))
        for g in range(NG)])
    iB = np.stack([
        _wrap16(slots_idx[g * GT:(g + 1) * GT, B0:CPT, :].reshape(-1))
        for g in range(NG)])
    iED = np.stack([
        _wrap16(slots_ed[g * GT:(g + 1) * GT, :, :].reshape(-1))
        for g in range(NG)])
    # dstrel as [NG, 128, GT*CPT] bf16 (partition-major per subchunk col)
    drel = slots_drel.reshape(NG, GT * CPT, 128).transpose(0, 2, 1)
    return iA, iB, iED, np.ascontiguousarray(drel.astype(ml_dtypes.bfloat16))


def _preprocess(edge_index):
    ei = np.asarray(edge_index).astype(np.int64)
    src = np.concatenate([ei[0], np.arange(N, dtype=np.int64)])
    dst = np.concatenate([ei[1], np.arange(N, dtype=np.int64)])
    order = np.argsort(dst, kind="stable")
    src, dst = src[order], dst[order]
    per_core = []
    for i in range(NCORES):
        m = (dst >= i * NP) & (dst < (i + 1) * NP)
        s = src[m]
        dl = dst[m] - i * NP
        tl = dl // 128
        drel = (dl % 128).astype(np.float32)
        # conv1: table = x rows (global node id)
        key1 = (s >= HALF).astype(np.int64)
        p1 = _edge_plan(s, drel, tl, key1, s)
        # conv2/3: table = allgathered he rows, chunk-block layout
        c = s // NP
        r = s % NP
        r2 = (r // CH) * (NCORES * CH) + c * CH + (r % CH)
        key2 = (r2 >= HALF).astype(np.int64)
        p2 = _edge_plan(s, drel, tl, key2, r2)
        per_core.append((p1, p2))
    return per_core


# ---------------------------------------------------------------- program
def _emit_prologue(nc, tc, cp, D):
    """Constants + weight prep + x_pad build + ed1 table."""
    sb = cp  # const pool
    C = {}
    ident_f = sb.tile([128, 128], f32, name="ident_f")
    make_identity(nc, ident_f[:])
    ident_b = sb.tile([128, 128], bf16, name="ident_b")
    nc.vector.tensor_copy(out=ident_b[:], in_=ident_f[:])
    iota_i = sb.tile([128, 128], mybir.dt.int32, name="iota_i")
    nc.gpsimd.iota(iota_i[:], pattern=[[1, 128]], base=0, channel_multiplier=0)
    iota_b = sb.tile([128, 128], bf16, name="iota_b")
    nc.vector.tensor_copy(out=iota_b[:], in_=iota_i[:])
    C["ident_f"], C["ident_b"], C["iota_b"] = ident_f, ident_b, iota_b

    # W1 [66, 1024] resident f32
    w1_sb = sb.tile([F_IN, H * C1], f32, name="w1_sb")
    nc.sync.dma_start(out=w1_sb[:], in_=D["W1"][:, :])
    C["w1_sb"] = w1_sb
    # b1T [128, 8] f32: column h = b1[h*128:(h+1)*128]
    b1T = sb.tile([128, H], f32, name="b1T")
    nc.sync.dma_start(out=b1T[:], in_=D["b1"][:].rearrange("(h c) -> c h", c=128))
    C["b1T"] = b1T

    with tc.tile_pool(name="prol_ps", bufs=3, space="PSUM") as pp, \
         tc.tile_pool(name="prol_sb", bufs=2) as ps:
        # as1/ad1 [8, 128] -> transposed [128, 8]
        a1s = ps.tile([H, C1], f32, name="a1s")
        nc.sync.dma_start(out=a1s[:], in_=D["as1"][:, :])
        a1d = ps.tile([H, C1], f32, name="a1d")
        nc.sync.dma_start(out=a1d[:], in_=D["ad1"][:, :])
        a1sT_ps = pp.tile([C1, H], f32, name="a1sT_ps", tag="pps", space="PSUM")
        nc.tensor.transpose(a1sT_ps[:], a1s[:], C["ident_f"][0:H, 0:H])
        a1dT_ps = pp.tile([C1, H], f32, name="a1dT_ps", tag="pps", space="PSUM")
        nc.tensor.transpose(a1dT_ps[:], a1d[:], C["ident_f"][0:H, 0:H])
        a1sT = ps.tile([C1, H], f32, name="a1sT")
        nc.vector.tensor_copy(out=a1sT[:], in_=a1sT_ps[:])
        a1dT = ps.tile([C1, H], f32, name="a1dT")
        nc.vector.tensor_copy(out=a1dT[:], in_=a1dT_ps[:])

        # wtilde1 src/dst [66, 8]: per head transpose W1_h then matmul
        wt1s = sb.tile([F_IN, H], bf16, name="wt1s")
        wt1d = sb.tile([F_IN, H], f32, name="wt1d")
        C["wt1s"], C["wt1d"] = wt1s, wt1d
        for h in range(H):
            w1hT_ps = pp.tile([C1, F_IN], f32, name="w1hT_ps", tag="pps", space="PSUM")
            nc.tensor.transpose(
                w1hT_ps[:], w1_sb[:, h * C1:(h + 1) * C1],
                C["ident_f"][0:F_IN, 0:F_IN])
            w1hT = ps.tile([C1, F_IN], f32, name="w1hT")
            nc.vector.tensor_copy(out=w1hT[:], in_=w1hT_ps[:])
            wt_ps = pp.tile([F_IN, 2], f32, name="wt_ps", tag="pps", space="PSUM")
            nc.tensor.matmul(wt_ps[:, 0:1], lhsT=w1hT[:], rhs=a1sT[:, h:h + 1],
                             start=True, stop=True)
            nc.tensor.matmul(wt_ps[:, 1:2], lhsT=w1hT[:], rhs=a1dT[:, h:h + 1],
                             start=True, stop=True)
            nc.vector.tensor_copy(out=wt1s[:, h:h + 1], in_=wt_ps[:, 0:1])
            nc.vector.tensor_copy(out=wt1d[:, h:h + 1], in_=wt_ps[:, 1:2])

        # a2 / a3 transposed columns
        a2 = ps.tile([2, C2], f32, name="a2")
        nc.sync.dma_start(out=a2[0:1, :], in_=D["as2"][:, :])
        nc.sync.dma_start(out=a2[1:2, :], in_=D["ad2"][:, :])
        a2T_ps = pp.tile([C2, 2], f32, name="a2T_ps", tag="pps", space="PSUM")
        nc.tensor.transpose(a2T_ps[:], a2[:], C["ident_f"][0:2, 0:2])
        a2T = ps.tile([C2, 2], f32, name="a2T")
        nc.vector.tensor_copy(out=a2T[:], in_=a2T_ps[:])
        a2sT_b = sb.tile([C2, 1], bf16, name="a2sT_b")
        nc.vector.tensor_copy(out=a2sT_b[:], in_=a2T[:, 0:1])
        C["a2sT_b"] = a2sT_b

        a3 = ps.tile([2, C3], f32, name="a3")
        nc.sync.dma_start(out=a3[0:1, :], in_=D["as3"][:, :])
        nc.sync.dma_start(out=a3[1:2, :], in_=D["ad3"][:, :])
        a3T_ps = pp.tile([C3, 2], f32, name="a3T_ps", tag="pps", space="PSUM")
        nc.tensor.transpose(a3T_ps[:], a3[:], C["ident_f"][0:2, 0:2])
        a3T = ps.tile([C3, 2], f32, name="a3T")
        nc.vector.tensor_copy(out=a3T[:], in_=a3T_ps[:])
        a3sT = sb.tile([C3, 1], f32, name="a3sT")
        nc.vector.tensor_copy(out=a3sT[:], in_=a3T[:, 0:1])
        C["a3sT"] = a3sT

        # W2aug [128, 129] bf16 x8 (col 128 = wtilde2_dst slice)
        w2aug = []
        for h in range(H):
            wa = sb.tile([C1, C2 + 1], bf16, name=f"w2aug{h}")
            w2s = ps.tile([C1, C2], f32, name="w2s")
            nc.sync.dma_start(out=w2s[:], in_=D["W2"][h * C1:(h + 1) * C1, :])
            nc.vector.tensor_copy(out=wa[:, 0:C2], in_=w2s[:])
            w2sT_ps = pp.tile([C2, C1], f32, name="w2sT_ps", tag="pps", space="PSUM")
            nc.tensor.transpose(w2sT_ps[:], w2s[:], C["ident_f"][:, :])
            w2sT = ps.tile([C2, C1], f32, name="w2sT")
            nc.vector.tensor_copy(out=w2sT[:], in_=w2sT_ps[:])
            wtd_ps = pp.tile([C1, 1], f32, name="wtd_ps", tag="pps", space="PSUM")
            nc.tensor.matmul(wtd_ps[:], lhsT=w2sT[:], rhs=a2T[:, 1:2],
                             start=True, stop=True)
            nc.vector.tensor_copy(out=wa[:, C2:C2 + 1], in_=wtd_ps[:])
            w2aug.append(wa)
        C["w2aug"] = w2aug

        # W3aug [128, 65] bf16 (col 64 = wtilde3_dst)
        w3aug = sb.tile([C2, C3 + 1], bf16, name="w3aug")
        w3s = ps.tile([C2, C3], f32, name="w3s")
        nc.sync.dma_start(out=w3s[:], in_=D["W3"][:, :])
        nc.vector.tensor_copy(out=w3aug[:, 0:C3], in_=w3s[:])
        w3sT_ps = pp.tile([C3, C2], f32, name="w3sT_ps", tag="pps", space="PSUM")
        nc.tensor.transpose(w3sT_ps[:], w3s[:], C["ident_f"][:, :])
        w3sT = ps.tile([C3, C2], f32, name="w3sT")
        nc.vector.tensor_copy(out=w3sT[:], in_=w3sT_ps[:])
        wtd3_ps = pp.tile([C2, 1], f32, name="wtd3_ps", tag="pps", space="PSUM")
        nc.tensor.matmul(wtd3_ps[:], lhsT=w3sT[:], rhs=a3T[:, 1:2],
                         start=True, stop=True)
        nc.vector.tensor_copy(out=w3aug[:, C3:C3 + 1], in_=wtd3_ps[:])
        C["w3aug"] = w3aug

        # b2bc [128, 128], b3bc [128, 64] via ones-matmul; br_bc [128, 1]
        ones = ps.tile([1, 128], f32, name="ones")
        nc.vector.memset(ones[:], 1.0)
        b2r = ps.tile([1, C2], f32, name="b2r")
        nc.sync.dma_start(out=b2r[:], in_=D["b2"][None, :])
        b2bc_ps = pp.tile([128, C2], f32, name="b2bc_ps", tag="pps", space="PSUM")
        nc.tensor.matmul(b2bc_ps[:], lhsT=ones[:], rhs=b2r[:], start=True, stop=True)
        b2bc = sb.tile([128, C2], f32, name="b2bc")
        nc.vector.tensor_copy(out=b2bc[:], in_=b2bc_ps[:])
        C["b2bc"] = b2bc
        b3r = ps.tile([1, C3], f32, name="b3r")
        nc.sync.dma_start(out=b3r[:], in_=D["b3"][None, :])
        b3bc_ps = pp.tile([128, C3], f32, name="b3bc_ps", tag="pps", space="PSUM")
        nc.tensor.matmul(b3bc_ps[:], lhsT=ones[:], rhs=b3r[:], start=True, stop=True)
        b3bc = sb.tile([128, C3], f32, name="b3bc")
        nc.vector.tensor_copy(out=b3bc[:], in_=b3bc_ps[:])
        C["b3bc"] = b3bc
        brr = ps.tile([1, 1], f32, name="brr")
        nc.sync.dma_start(out=brr[:], in_=D["br"][None, :])
        brbc_ps = pp.tile([128, 1], f32, name="brbc_ps", tag="pps", space="PSUM")
        nc.tensor.matmul(brbc_ps[:], lhsT=ones[:], rhs=brr[:], start=True, stop=True)
        br_bc = sb.tile([128, 1], f32, name="br_bc")
        nc.vector.tensor_copy(out=br_bc[:], in_=brbc_ps[:])
        C["br_bc"] = br_bc
        wr = sb.tile([C3, 1], f32, name="wr")
        nc.sync.dma_start(out=wr[:], in_=D["Wr"][:, :])
        C["wr"] = wr

        # neginf row for ed pad entries
        neg = ps.tile([1, 64], f32, name="neg")
        nc.vector.memset(neg[:], -1e30)
        for ed in (D["ed1"], D["ed2"], D["ed3"]):
            nc.sync.dma_start(out=ed[NPAD:NPAD + 1, 0:64], in_=neg[:])

    # x_pad build: bf16 [50000, 128] (tail cols garbage, never read)
    with tc.tile_pool(name="xp_sb", bufs=3) as xp:
        CH_R = 512
        nfull = N // CH_R           # 97
        for ci in range(nfull + 1):
            r0 = ci * CH_R
            rows = min(CH_R, N - r0)
            blocks = rows // 128
            if blocks:
                xin = xp.tile([128, blocks * F_IN], f32, tag="xin")
                nc.sync.dma_start(
                    out=xin[:].rearrange("p (c f) -> p c f", f=F_IN),
                    in_=D["x"][r0:r0 + blocks * 128, :]
                    .rearrange("(c p) f -> p c f", p=128))
                xob = xp.tile([128, blocks * F_IN], bf16, tag="xob")
                nc.vector.tensor_copy(out=xob[:], in_=xin[:])
                nc.sync.dma_start(
                    out=AP(tensor=D["x_pad"][:, :].tensor, offset=r0 * 128,
                           ap=[[128, 128], [128 * 128, blocks], [1, F_IN]]),
                    in_=xob[:].rearrange("p (c f) -> p c f", f=F_IN))
            rem = rows - blocks * 128
            if rem:
                r1 = r0 + blocks * 128
                xin2 = xp.tile([rem, F_IN], f32, tag="xin2")
                nc.sync.dma_start(out=xin2[:], in_=D["x"][r1:r1 + rem, :])
                xob2 = xp.tile([rem, F_IN], bf16, tag="xob2")
                nc.vector.tensor_copy(out=xob2[:], in_=xin2[:])
                nc.sync.dma_start(
                    out=AP(tensor=D["x_pad"][:, :].tensor, offset=r1 * 128,
                           ap=[[128, rem], [1, F_IN]]),
                    in_=xob2[:])

    # ed1 table: e1dst for own nodes, per tile
    with tc.tile_pool(name="ed1_ps", bufs=2, space="PSUM") as pp, \
         tc.tile_pool(name="ed1_sb", bufs=3) as ps:
        for t in range(T):
            xo = ps.tile([128, F_IN], f32, tag="xo")
            nc.sync.dma_start(out=xo[:], in_=D["x_own"][t * 128:(t + 1) * 128, :])
            xoT_ps = pp.tile([F_IN, 128], f32, tag="xoT_ps", space="PSUM")
            nc.tensor.transpose(xoT_ps[:], xo[:], C["ident_f"][:, :])
            xoT = ps.tile([F_IN, 128], f32, tag="xoT")
            nc.vector.tensor_copy(out=xoT[:], in_=xoT_ps[:])
            ed_ps = pp.tile([128, H], f32, tag="ed_ps", space="PSUM")
            nc.tensor.matmul(ed_ps[:], lhsT=xoT[:], rhs=C["wt1d"][:, :],
                             start=True, stop=True)
            ed_sb = ps.tile([128, H], f32, tag="ed_sb")
            nc.vector.tensor_copy(out=ed_sb[:], in_=ed_ps[:])
            nc.sync.dma_start(out=D["ed1"][t * 128:(t + 1) * 128, 0:H], in_=ed_sb[:])
    return C


def _gather_group(nc, pool, D, g, conv, table, ed_tab, elem, estep, dt):
    """Issue the three dma_gathers for group g. Returns (gA, gB, gED)."""
    pfx = f"c{conv}"
    nA, nB, nE = GT * B0 * 128, GT * B1 * 128, GT * CPT * 128
    iA = pool.tile([128, nA // 16], i16, tag=f"{pfx}iA")
    nc.sync.dma_start(out=iA[:], in_=D[f"{pfx}_iA"][g, :, :])
    iB = pool.tile([128, nB // 16], i16, tag=f"{pfx}iB")
    nc.sync.dma_start(out=iB[:], in_=D[f"{pfx}_iB"][g, :, :])
    iE = pool.tile([128, nE // 16], i16, tag=f"{pfx}iE")
    nc.sync.dma_start(out=iE[:], in_=D[f"{pfx}_iED"][g, :, :])

    ta = table[:, :]
    gA = pool.tile([128, GT * B0 * elem], dt, tag=f"{pfx}gA")
    nc.gpsimd.dma_gather(
        out_ap=gA[:].rearrange("p (k d) -> p k d", d=elem),
        in_ap=AP(tensor=ta.tensor, offset=0, ap=[[estep, HALF], [1, elem]]),
        idxs_ap=iA[:], num_idxs=nA, num_idxs_reg=nA,
        elem_size=elem, elem_step=estep, single_packet=False)
    gB = pool.tile([128, GT * B1 * elem], dt, tag=f"{pfx}gB")
    nrows_b = (table.shape[0] if hasattr(table, "shape") else NHE) - HALF
    nc.gpsimd.dma_gather(
        out_ap=gB[:].rearrange("p (k d) -> p k d", d=elem),
        in_ap=AP(tensor=ta.tensor, offset=HALF * estep,
                 ap=[[estep, nrows_b], [1, elem]]),
        idxs_ap=iB[:], num_idxs=nB, num_idxs_reg=nB,
        elem_size=elem, elem_step=estep, single_packet=False)
    te = ed_tab[:, :]
    gE = pool.tile([128, GT * CPT * 64], f32, tag=f"{pfx}gE")
    nc.gpsimd.dma_gather(
        out_ap=gE[:].rearrange("p (k d) -> p k d", d=64),
        in_ap=AP(tensor=te.tensor, offset=0, ap=[[64, EDR], [1, 64]]),
        idxs_ap=iE[:], num_idxs=nE, num_idxs_reg=nE,
        elem_size=64, elem_step=64, single_packet=False)
    drt = pool.tile([128, GT * CPT], bf16, tag=f"{pfx}dr")
    nc.sync.dma_start(out=drt[:], in_=D[f"{pfx}_drel"][g, :, :])
    return gA, gB, gE, drt


def _edge_w(nc, wp, esrc_ps, gE, col, nh, tag, wdt):
    """w = exp(leakyrelu(esrc + edst)) -> [128, nh] in dtype wdt."""
    es = wp.tile([128, nh], f32, tag=f"{tag}es")
    nc.vector.tensor_tensor(out=es[:], in0=esrc_ps[:],
                            in1=gE[:, col:col + nh], op=OP.add)
    lr = wp.tile([128, nh], f32, tag=f"{tag}lr")
    nc.vector.scalar_tensor_tensor(out=lr[:], in0=es[:], scalar=NEG_SLOPE,
                                   in1=es[:], op0=OP.mult, op1=OP.max)
    w = wp.tile([128, nh], wdt, tag=f"{tag}w")
    nc.scalar.activation(w[:], lr[:], AF.Exp)
    return w


def _build():
    nc = bacc.Bacc("TRN2", target_bir_lowering=False, debug=False,
                   num_devices=NCORES)
    D = {}
    # inputs
    D["x"] = nc.dram_tensor("x", [N, F_IN], f32, kind="ExternalInput")
    D["x_own"] = nc.dram_tensor("x_own", [NPAD, F_IN], f32, kind="ExternalInput")
    for nm, shp, dt in [
        ("W1", [F_IN, H * C1], f32), ("b1", [H * C1], f32),
        ("as1", [H, C1], f32), ("ad1", [H, C1], f32),
        ("W2", [H * C1, C2], f32), ("b2", [C2], f32),
        ("as2", [1, C2], f32), ("ad2", [1, C2], f32),
        ("W3", [C2, C3], f32), ("b3", [C3], f32),
        ("as3", [1, C3], f32), ("ad3", [1, C3], f32),
        ("Wr", [C3, 1], f32), ("br", [1], f32),
    ]:
        D[nm] = nc.dram_tensor(nm, shp, dt, kind="ExternalInput")
    for conv, half_n in (("c1", (GT * B0 * 128, GT * B1 * 128)),
                         ("c2", (GT * B0 * 128, GT * B1 * 128))):
        nA, nB = half_n
        nE = GT * CPT * 128
        D[f"{conv}_iA"] = nc.dram_tensor(f"{conv}_iA", [NG, 128, nA // 16], i16,
                                         kind="ExternalInput")
        D[f"{conv}_iB"] = nc.dram_tensor(f"{conv}_iB", [NG, 128, nB // 16], i16,
                                         kind="ExternalInput")
        D[f"{conv}_iED"] = nc.dram_tensor(f"{conv}_iED", [NG, 128, nE // 16], i16,
                                          kind="ExternalInput")
        D[f"{conv}_drel"] = nc.dram_tensor(f"{conv}_drel", [NG, 128, GT * CPT],
                                           bf16, kind="ExternalInput")
    # outputs
    D["y_out"] = nc.dram_tensor("y_out", [NPAD, 1], f32, kind="ExternalOutput")
    # scratch
    D["x_pad"] = nc.dram_tensor("x_pad", [N, 128], bf16, kind="Internal")
    D["ed1"] = nc.dram_tensor("ed1", [EDR, 64], f32, kind="Internal")
    D["ed2"] = nc.dram_tensor("ed2", [EDR, 64], f32, kind="Internal")
    D["ed3"] = nc.dram_tensor("ed3", [EDR, 64], f32, kind="Internal")
    D["he2_loc"] = nc.dram_tensor("he2_loc", [NPAD, C2], bf16, kind="Internal")
    D["he3_loc"] = nc.dram_tensor("he3_loc", [NPAD, C3], f32, kind="Internal")
    D["he2"] = nc.dram_tensor("he2", [NHE, C2], bf16, kind="Internal",
                              addr_space="Shared")
    D["he3"] = nc.dram_tensor("he3", [NHE, C3], f32, kind="Internal",
                              addr_space="Shared")
    RG = [list(range(NCORES))]

    with tile.TileContext(nc) as tc:
        with tc.tile_pool(name="const", bufs=1) as cp:
            C = _emit_prologue(nc, tc, cp, D)

            # ---------------- conv1 ----------------
            with tc.tile_pool(name="c1_ps", bufs=1, space="PSUM") as pp, \
                 tc.tile_pool(name="c1_gb", bufs=2) as gb, \
                 tc.tile_pool(name="c1_wk", bufs=3) as wk:
                for g in range(NG):
                    gA, gB, gE, drt = _gather_group(
                        nc, gb, D, g, 1, D["x_pad"], D["ed1"], 128, 128, bf16)
                    for ti in range(GT):
                        t = g * GT + ti
                        pA = pp.tile([128, 268], f32, tag="pA", space="PSUM")
                        pB = pp.tile([128, 268], f32, tag="pB", space="PSUM")
                        for s in range(CPT):
                            if s < B0:
                                xc = gA[:, (ti * B0 + s) * 128:(ti * B0 + s) * 128 + 128]
                            else:
                                xc = gB[:, (ti * B1 + s - B0) * 128:
                                        (ti * B1 + s - B0) * 128 + 128]
                            # esrc on the fly
                            xcT_ps = pp.tile([F_IN, 128], bf16, tag="tp_ps",
                                             space="PSUM", bufs=2)
                            nc.tensor.transpose(xcT_ps[:], xc[:, 0:F_IN],
                                                C["ident_b"][:, :])
                            xcT = wk.tile([F_IN, 128], bf16, tag="xcT")
                            nc.scalar.copy(out=xcT[:], in_=xcT_ps[:])
                            es_ps = pp.tile([128, H], f32, tag="mm_ps",
                                            space="PSUM", bufs=2)
                            nc.tensor.matmul(es_ps[:], lhsT=xcT[:],
                                             rhs=C["wt1s"][:, :], start=True,
                                             stop=True)
                            ecol = (ti * CPT + s) * 64
                            w = _edge_w(nc, wk, es_ps, gE, ecol, H, "c1", f32)
                            # sel
                            sel = wk.tile([128, 128], bf16, tag="sel")
                            nc.vector.tensor_tensor(
                                out=sel[:], in0=C["iota_b"][:, :],
                                in1=drt[:, ti * CPT + s:ti * CPT + s + 1]
                                .to_broadcast([128, 128]), op=OP.is_equal)
                            # msg tiles
                            mA = wk.tile([128, 268], bf16, tag="mA")
                            mB = wk.tile([128, 268], bf16, tag="mB")
                            for h in range(H):
                                dstt = mA if h < 4 else mB
                                off = (h % 4) * F_IN
                                if h % 2 == 0:
                                    nc.vector.tensor_scalar_mul(
                                        dstt[:, off:off + F_IN], xc[:, 0:F_IN],
                                        w[:, h:h + 1])
                                else:
                                    nc.scalar.mul(dstt[:, off:off + F_IN],
                                                  xc[:, 0:F_IN], w[:, h:h + 1])
                            nc.vector.tensor_copy(out=mA[:, 264:268], in_=w[:, 0:4])
                            nc.vector.tensor_copy(out=mB[:, 264:268], in_=w[:, 4:8])
                            nc.tensor.matmul(pA[:], lhsT=sel[:], rhs=mA[:],
                                             start=(s == 0), stop=(s == CPT - 1))
                            nc.tensor.matmul(pB[:], lhsT=sel[:], rhs=mB[:],
                                             start=(s == 0), stop=(s == CPT - 1))
                        # ---- tile epilogue ----
                        z = wk.tile([128, H], f32, tag="z")
                        nc.vector.tensor_copy(out=z[:, 0:4], in_=pA[:, 264:268])
                        nc.vector.tensor_copy(out=z[:, 4:8], in_=pB[:, 264:268])
                        rz = wk.tile([128, H], f32, tag="rz")
                        nc.vector.reciprocal(out=rz[:], in_=z[:])
                        stage = wk.tile([128, H * C1], bf16, tag="stage")
                        h2e = pp.tile([128, C2 + 1], f32, tag="h2e", space="PSUM")
                        for h in range(H):
                            src_ps = pA if h < 4 else pB
                            off = (h % 4) * F_IN
                            gn = wk.tile([128, F_IN], f32, tag="gn", bufs=4)
                            nc.vector.tensor_scalar_mul(
                                gn[:], src_ps[:, off:off + F_IN], rz[:, h:h + 1])
                            gnT_ps = pp.tile([F_IN, 128], f32, tag="tp_ps",
                                             space="PSUM", bufs=2)
                            nc.tensor.transpose(gnT_ps[:], gn[:], C["ident_f"][:, :])
                            gnT = wk.tile([F_IN, 128], f32, tag="gnT")
                            nc.scalar.copy(out=gnT[:], in_=gnT_ps[:])
                            o1_ps = pp.tile([128, 128], f32, tag="mm_ps",
                                            space="PSUM", bufs=2)
                            nc.tensor.matmul(
                                o1_ps[:], lhsT=C["w1_sb"][:, h * C1:(h + 1) * C1],
                                rhs=gnT[:], start=True, stop=True)
                            nc.scalar.activation(
                                stage[:, h * C1:(h + 1) * C1], o1_ps[:],
                                AF.Identity, bias=C["b1T"][:, h:h + 1])
                        mst = wk.tile([128, H * C1], bf16, tag="mst")
                        nc.vector.tensor_scalar_min(mst[:], stage[:], 0.0)
                        pst = wk.tile([128, H * C1], bf16, tag="pst")
                        nc.scalar.activation(pst[:], mst[:], AF.Exp)
                        elu = wk.tile([128, H * C1], bf16, tag="elu")
                        nc.vector.scalar_tensor_tensor(
                            out=elu[:], in0=pst[:], scalar=-1.0, in1=stage[:],
                            op0=OP.add, op1=OP.max)
                        for h in range(H):
                            nc.tensor.matmul(
                                h2e[:], lhsT=elu[:, h * C1:(h + 1) * C1],
                                rhs=C["w2aug"][h][:, :], start=(h == 0),
                                stop=(h == H - 1))
                        h2sb = wk.tile([128, C2], bf16, tag="h2sb")
                        nc.vector.tensor_copy(out=h2sb[:], in_=h2e[:, 0:C2])
                        nc.sync.dma_start(
                            out=D["he2_loc"][t * 128:(t + 1) * 128, :], in_=h2sb[:])
                        e2d = wk.tile([128, 1], f32, tag="e2d")
                        nc.vector.tensor_copy(out=e2d[:], in_=h2e[:, C2:C2 + 1])
                        nc.sync.dma_start(
                            out=D["ed2"][t * 128:(t + 1) * 128, 0:1], in_=e2d[:])
                    # allgather chunk g of he2
                    nc.gpsimd.collective_compute(
                        "AllGather", OP.bypass, replica_groups=RG,
                        ins=[D["he2_loc"][g * CH:(g + 1) * CH, :]],
                        outs=[D["he2"][g * NCORES * CH:(g + 1) * NCORES * CH, :]])

            # ---------------- conv2 ----------------
            with tc.tile_pool(name="c2_ps", bufs=1, space="PSUM") as pp, \
                 tc.tile_pool(name="c2_gb", bufs=2) as gb, \
                 tc.tile_pool(name="c2_wk", bufs=3) as wk:
                for g in range(NG):
                    gA, gB, gE, drt = _gather_group(
                        nc, gb, D, g, 2, D["he2"], D["ed2"], 128, 128, bf16)
                    for ti in range(GT):
                        t = g * GT + ti
                        g2 = pp.tile([128, C2 + 1], f32, tag="g2", space="PSUM")
                        for s in range(CPT):
                            if s < B0:
                                hc = gA[:, (ti * B0 + s) * 128:(ti * B0 + s) * 128 + 128]
                            else:
                                hc = gB[:, (ti * B1 + s - B0) * 128:
                                        (ti * B1 + s - B0) * 128 + 128]
                            hcT_ps = pp.tile([C2, 128], bf16, tag="hcT_ps",
                                             space="PSUM", bufs=2)
                            nc.tensor.transpose(hcT_ps[:], hc[:], C["ident_b"][:, :])
                            hcT = wk.tile([C2, 128], bf16, tag="hcT")
                            nc.scalar.copy(out=hcT[:], in_=hcT_ps[:])
                            es_ps = pp.tile([128, 1], f32, tag="es2_ps",
                                            space="PSUM", bufs=2)
                            nc.tensor.matmul(es_ps[:], lhsT=hcT[:],
                                             rhs=C["a2sT_b"][:, :], start=True,
                                             stop=True)
                            ecol = (ti * CPT + s) * 64
                            w = _edge_w(nc, wk, es_ps, gE, ecol, 1, "c2", f32)
                            sel = wk.tile([128, 128], bf16, tag="sel2")
                            nc.vector.tensor_tensor(
                                out=sel[:], in0=C["iota_b"][:, :],
                                in1=drt[:, ti * CPT + s:ti * CPT + s + 1]
                                .to_broadcast([128, 128]), op=OP.is_equal)
                            msg = wk.tile([128, C2 + 1], bf16, tag="msg2")
                            nc.vector.tensor_scalar_mul(msg[:, 0:C2], hc[:],
                                                        w[:, 0:1])
                            nc.vector.tensor_copy(out=msg[:, C2:C2 + 1], in_=w[:])
                            nc.tensor.matmul(g2[:], lhsT=sel[:], rhs=msg[:],
                                             start=(s == 0), stop=(s == CPT - 1))
                        # epilogue
                        rz = wk.tile([128, 1], f32, tag="rz2")
                        nc.vector.reciprocal(out=rz[:], in_=g2[:, C2:C2 + 1])
                        s2 = wk.tile([128, C2], bf16, tag="s2")
                        nc.vector.scalar_tensor_tensor(
                            out=s2[:], in0=g2[:, 0:C2], scalar=rz[:, 0:1],
                            in1=C["b2bc"][:, :], op0=OP.mult, op1=OP.add)
                        m2 = wk.tile([128, C2], bf16, tag="m2")
                        nc.vector.tensor_scalar_min(m2[:], s2[:], 0.0)
                        p2 = wk.tile([128, C2], bf16, tag="p2")
                        nc.scalar.activation(p2[:], m2[:], AF.Exp)
                        el2 = wk.tile([128, C2], bf16, tag="el2")
                        nc.vector.scalar_tensor_tensor(
                            out=el2[:], in0=p2[:], scalar=-1.0, in1=s2[:],
                            op0=OP.add, op1=OP.max)
                        el2T_ps = pp.tile([C2, 128], bf16, tag="el2T_ps",
                                          space="PSUM", bufs=2)
                        nc.tensor.transpose(el2T_ps[:], el2[:], C["ident_b"][:, :])
                        el2T = wk.tile([C2, 128], bf16, tag="el2T")
                        nc.scalar.copy(out=el2T[:], in_=el2T_ps[:])
                        h3e_ps = pp.tile([128, C3 + 1], f32, tag="h3e", space="PSUM")
                        nc.tensor.matmul(h3e_ps[:], lhsT=el2T[:],
                                         rhs=C["w3aug"][:, :], start=True, stop=True)
                        h3f = wk.tile([128, C3 + 1], f32, tag="h3f")
                        nc.vector.tensor_copy(out=h3f[:], in_=h3e_ps[:])
                        nc.sync.dma_start(
                            out=D["he3_loc"][t * 128:(t + 1) * 128, :],
                            in_=h3f[:, 0:C3])
                        nc.sync.dma_start(
                            out=D["ed3"][t * 128:(t + 1) * 128, 0:1],
                            in_=h3f[:, C3:C3 + 1])
                    nc.gpsimd.collective_compute(
                        "AllGather", OP.bypass, replica_groups=RG,
                        ins=[D["he3_loc"][g * CH:(g + 1) * CH, :]],
                        outs=[D["he3"][g * NCORES * CH:(g + 1) * NCORES * CH, :]])

            # ---------------- conv3 + regressor ----------------
            with tc.tile_pool(name="c3_ps", bufs=1, space="PSUM") as pp, \
                 tc.tile_pool(name="c3_gb", bufs=2) as gb, \
                 tc.tile_pool(name="c3_wk", bufs=3) as wk:
                for g in range(NG):
                    gA, gB, gE, drt = _gather_group(
                        nc, gb, D, g, 2, D["he3"], D["ed3"], C3, C3, f32)
                    for ti in range(GT):
                        t = g * GT + ti
                        g3 = pp.tile([128, C3 + 1], f32, tag="g3", space="PSUM")
                        for s in range(CPT):
                            if s < B0:
                                hc = gA[:, (ti * B0 + s) * C3:(ti * B0 + s) * C3 + C3]
                            else:
                                hc = gB[:, (ti * B1 + s - B0) * C3:
                                        (ti * B1 + s - B0) * C3 + C3]
                            hcT_ps = pp.tile([C3, 128], f32, tag="hcT3_ps",
                                             space="PSUM", bufs=2)
                            nc.tensor.transpose(hcT_ps[:], hc[:], C["ident_f"][:, :])
                            hcT = wk.tile([C3, 128], f32, tag="hcT3")
                            nc.scalar.copy(out=hcT[:], in_=hcT_ps[:])
                            es_ps = pp.tile([128, 1], f32, tag="es3_ps",
                                            space="PSUM", bufs=2)
                            nc.tensor.matmul(es_ps[:], lhsT=hcT[:],
                                             rhs=C["a3sT"][:, :], start=True,
                                             stop=True)
                            ecol = (ti * CPT + s) * 64
                            w = _edge_w(nc, wk, es_ps, gE, ecol, 1, "c3", f32)
                            sel = wk.tile([128, 128], bf16, tag="sel3")
                            nc.vector.tensor_tensor(
                                out=sel[:], in0=C["iota_b"][:, :],
                                in1=drt[:, ti * CPT + s:ti * CPT + s + 1]
                                .to_broadcast([128, 128]), op=OP.is_equal)
                            msg = wk.tile([128, C3 + 1], bf16, tag="msg3")
                            nc.vector.tensor_scalar_mul(msg[:, 0:C3], hc[:],
                                                        w[:, 0:1])
                            nc.vector.tensor_copy(out=msg[:, C3:C3 + 1], in_=w[:])
                            nc.tensor.matmul(g3[:], lhsT=sel[:], rhs=msg[:],
                                             start=(s == 0), stop=(s == CPT - 1))
                        rz = wk.tile([128, 1], f32, tag="rz3")
                        nc.vector.reciprocal(out=rz[:], in_=g3[:, C3:C3 + 1])
                        s3 = wk.tile([128, C3], f32, tag="s3")
                        nc.vector.scalar_tensor_tensor(
                            out=s3[:], in0=g3[:, 0:C3], scalar=rz[:, 0:1],
                            in1=C["b3bc"][:, :], op0=OP.mult, op1=OP.add)
                        m3 = wk.tile([128, C3], f32, tag="m3")
                        nc.vector.tensor_scalar_min(m3[:], s3[:], 0.0)
                        p3 = wk.tile([128, C3], f32, tag="p3")
                        nc.scalar.activation(p3[:], m3[:], AF.Exp)
                        el3 = wk.tile([128, C3], f32, tag="el3")
                        nc.vector.scalar_tensor_tensor(
                            out=el3[:], in0=p3[:], scalar=-1.0, in1=s3[:],
                            op0=OP.add, op1=OP.max)
                        el3T_ps = pp.tile([C3, 128], f32, tag="fin_ps",
                                          space="PSUM", bufs=2)
                        nc.tensor.transpose(el3T_ps[:], el3[:], C["ident_f"][:, :])
                        el3T = wk.tile([C3, 128], f32, tag="el3T")
                        nc.scalar.copy(out=el3T[:], in_=el3T_ps[:])
                        y_ps = pp.tile([128, 1], f32, tag="fin_ps", space="PSUM",
                                       bufs=2)
                        nc.tensor.matmul(y_ps[:], lhsT=el3T[:], rhs=C["wr"][:, :],
                                         start=True, stop=True)
                        y_sb = wk.tile([128, 1], f32, tag="y_sb")
                        nc.scalar.activation(y_sb[:], y_ps[:], AF.Sigmoid,
                                             bias=C["br_bc"][:, 0:1])
                        nc.sync.dma_start(
                            out=D["y_out"][t * 128:(t + 1) * 128, :], in_=y_sb[:])
    nc.compile()
    return nc


def build_in_maps(inputs, plans):
    x = np.ascontiguousarray(np.asarray(inputs["x"], dtype=np.float32))
    in_maps = []
    for i in range(NCORES):
        (c1_iA, c1_iB, c1_iED, c1_dr), (c2_iA, c2_iB, c2_iED, c2_dr) = plans[i]
        xo = np.zeros((NPAD, F_IN), np.float32)
        xo[0:NP] = x[i * NP:(i + 1) * NP]
        m = {
            "x": x, "x_own": xo,
            "c1_iA": c1_iA, "c1_iB": c1_iB, "c1_iED": c1_iED, "c1_drel": c1_dr,
            "c2_iA": c2_iA, "c2_iB": c2_iB, "c2_iED": c2_iED, "c2_drel": c2_dr,
        }
        for nm in ("W1", "b1", "as1", "ad1", "W2", "b2", "as2", "ad2",
                   "W3", "b3", "as3", "ad3", "Wr", "br"):
            m[nm] = np.ascontiguousarray(np.asarray(inputs[nm], dtype=np.float32))
        m["Wr"] = m["Wr"].reshape(C3, 1)
        m["br"] = m["br"].reshape(1)
        m["as2"] = m["as2"].reshape(1, C2)
        m["ad2"] = m["ad2"].reshape(1, C2)
        m["as3"] = m["as3"].reshape(1, C3)
        m["ad3"] = m["ad3"].reshape(1, C3)
        in_maps.append(m)
    return in_maps


def kernel(**inputs):
    if "prog" not in _CACHE:
        _CACHE["prog"] = _build()
    nc = _CACHE["prog"]
    plans = _preprocess(inputs["edge_index"])
    in_maps = build_in_maps(inputs, plans)
    res = run_bass_kernel_spmd(nc, in_maps, core_ids=list(range(NCORES)))
    out = np.concatenate(
        [res.results[i]["y_out"][0:NP, 0] for i in range(NCORES)])
    return out.astype(np.float32)

